# revision 1
# baseline (speedup 1.0000x reference)
"""Trainium2 Bass kernel for nn_HKANGNN (hetero GraphConv + KAN head).

Math (only the email-node output path matters):
  e    = x_email @ w_email.T + b_email
  agg_se[n] = sum_{se edges -> n} (x_sender[src] @ w_sender.T + b_sender)
  agg_ue[n] = sum_{ue edges -> n} (x_url[src]    @ w_url.T    + b_url)
  out_e = agg_se @ w_rel_se.T + b_rel_se + agg_ue @ w_rel_ue.T + b_rel_ue
        + e @ (w_root_se + w_root_ue).T
  h = relu(out_e);  out = silu(h) @ base_w.T + einsum(b_splines(h), spline_w)

Device strategy (8 cores, email nodes sharded 12500/core, padded to 12800):
  * linearity lets the per-edge features be the RAW source features plus a
    count column; the tiny projection matrices fold into Mcomb on host.
  * segment-sum = one-hot matmuls accumulated in PSUM per 128-dst tile;
    per-edge rows fetched with dma_gather (3 source classes so int16 idx fit).
  * projection: out_e.T[h, n] accumulated in PSUM over 6 K-chunks of
    (Wrootsum@w_email).T as stationary weights (bf16).
  * KAN head: h>=0 and all bases vanish for h>=2.2, so with x = clamp(h,0,2.2)
    spline(h) == q0+q1 x+q2 x^2+q3 x^3 + sum_k W'_k relu(x-t_k)^3 (t_k=.2..1.8)
    -> 10 extra matmul K-chunks ([silu,1,x,x^2,x^3,R6^3..R10^3]) into a [2,512]
    PSUM per 512-node tile.
"""

import os
import numpy as np
import ml_dtypes

import concourse.bass as bass
import concourse.mybir as mybir
import concourse.tile as tile
from concourse import bacc
from concourse.bass_utils import run_bass_kernel_spmd

F32 = mybir.dt.float32
BF16 = mybir.dt.bfloat16
BF = ml_dtypes.bfloat16

N_CORES = 8
HID = 128
NE, NS, NU = 100000, 30000, 50000
NSH = NE // N_CORES          # 12500 real nodes per core
NP = 12800                   # padded (25 x 512 node tiles, 100 x 128 dst tiles)
NT128 = NP // 128            # 100 dst tiles
NT512 = NP // 512            # 25 node tiles
KIN = 768
NKC = KIN // 128             # 6 projection K-chunks
URL_SPLIT = 25600            # url class A rows [0,25600), B rows [25600,50000)
ELEM = 128                   # gather row: 128 bf16 = 256 B
CH_T = 10                    # dst tiles per gather chunk
N_CH = NT128 // CH_T         # 10 chunks
KNOTS = (0.2, 0.6, 1.0, 1.4, 1.8)
XCLAMP = 2.2

_LAST_RESULT = None
_CACHE = {}


# ----------------------------------------------------------------- host folds
def _head_weights(base_w, spline_w):
    """[128, 20] f32: lhsT ([d,2]) per head chunk, order
    [silu, ones, x, x^2, x^3, R(.2)^3, R(.6)^3, R(1.0)^3, R(1.4)^3, R(1.8)^3]."""
    c = np.array([1.0, -4.0, 6.0, -4.0, 1.0], np.float64)
    h = 0.4
    scale = 1.0 / (6.0 * h ** 3)
    O, D, B = spline_w.shape                      # [2, 128, 8]
    wp = np.zeros((O, D, 11), np.float64)         # W'[o,d,m], m=0..10
    for m in range(11):
        for j in range(5):
            b = m - j
            if 0 <= b < B:
                wp[:, :, m] += spline_w[:, :, b].astype(np.float64) * c[j] * scale
    t = np.arange(11) * h - 2.2                   # knot m at t_m
    q = np.zeros((4, O, D), np.float64)           # poly coeffs from m=0..5
    for m in range(6):
        q[0] += -t[m] ** 3 * wp[:, :, m]
        q[1] += 3 * t[m] ** 2 * wp[:, :, m]
        q[2] += -3 * t[m] * wp[:, :, m]
        q[3] += wp[:, :, m]
    head = np.zeros((D, 20), np.float64)
    head[:, 0:2] = base_w.T                       # silu chunk
    for j in range(4):                            # ones, x, x^2, x^3
        head[:, 2 * (1 + j):2 * (1 + j) + 2] = q[j].T
    for k in range(5):                            # relu^3 knots m=6..10
        head[:, 2 * (5 + k):2 * (5 + k) + 2] = wp[:, :, 6 + k].T
    return head.astype(np.float32)


def _fold_weights(inp):
    wrs = inp["w_root_se"] + inp["w_root_ue"]
    wbigT = (wrs @ inp["w_email"]).T.copy()                     # [768, 128]
    mcomb = np.zeros((12, 128), np.float32)
    mcomb[0] = inp["w_rel_se"] @ inp["w_sender"][:, 0]
    mcomb[1] = inp["w_rel_se"] @ inp["b_sender"]
    mcomb[2:10] = (inp["w_rel_ue"] @ inp["w_url"]).T
    mcomb[10] = inp["w_rel_ue"] @ inp["b_url"]
    mcomb[11] = inp["b_rel_se"] + inp["b_rel_ue"] + wrs @ inp["b_email"]
    head = _head_weights(inp["base_w"], inp["spline_w"])
    return wbigT, mcomb, head


def _wrap_idx16(flat):
    """int16 slot list -> [128, n/16] wrapped in 16 partitions, tiled to 128."""
    n = flat.shape[0]
    a = flat.astype(np.int16).reshape(n // 16, 16).T            # [16, n/16]
    return np.tile(a, (8, 1))


def _prep_edges(inp):
    """Per-core per-class slot arrays (idx into class tables + local dst)."""
    cls_edges = []
    # (src_in_class_table, dst_email) per class
    cls_edges.append((inp["se_src"], inp["se_dst"]))                     # S
    ua = inp["ue_src"] < URL_SPLIT
    cls_edges.append((inp["ue_src"][ua], inp["ue_dst"][ua]))             # A
    cls_edges.append((inp["ue_src"][~ua] - URL_SPLIT, inp["ue_dst"][~ua]))  # B
    zrow = (NS, URL_SPLIT, NU - URL_SPLIT)                               # zero-row ids
    percls = []
    for ci, (src, dst) in enumerate(cls_edges):
        per_core = []
        gmax = 1
        for c in range(N_CORES):
            sel = (dst >= c * NSH) & (dst < (c + 1) * NSH)
            s, d = src[sel], dst[sel] - c * NSH
            order = np.argsort(d, kind="stable")
            s, d = s[order], d[order]
            tlist = []
            for t in range(NT128):
                m = (d >= t * 128) & (d < (t + 1) * 128)
                tlist.append((s[m], d[m] - t * 128))
                gmax = max(gmax, (len(tlist[-1][0]) + 127) // 128)
            per_core.append(tlist)
        percls.append((per_core, gmax, zrow[ci]))
    out = []
    for per_core, gmax, zr in percls:
        idxs, dsts = [], []
        for c in range(N_CORES):
            slots = np.full((NT128, gmax * 128), zr, np.int32)
            dloc = np.zeros((NT128, gmax * 128), np.float32)
            for t, (s, d) in enumerate(per_core[c]):
                slots[t, : len(s)] = s
                dloc[t, : len(s)] = d
            # slot j within a group of 128 -> partition j%128; groups tile-major
            flat = slots.reshape(-1)                             # [100*gmax*128]
            idxs.append(_wrap_idx16(flat))
            oh = (dloc.reshape(NT128 * gmax, 128).T[:, :, None]
                  == np.arange(128)[None, None, :]).astype(BF)
            dsts.append(oh.reshape(128, NT128 * gmax * 128))   # [128, ngrp*128]
        out.append((np.stack(idxs), np.stack(dsts), gmax))
    return out  # [(idx16 [8,128,*], dst [8,128,100*g], g)] * 3


# ----------------------------------------------------------------- device build
def _build(gS, gA, gB):
    nc = bacc.Bacc("TRN2", target_bir_lowering=False, debug=False,
                   num_devices=N_CORES)
    dt = lambda n, s, d, k: nc.dram_tensor(n, s, d, kind=k).ap()
    xT = dt("xT", [KIN, NP], BF16, "ExternalInput")
    tabS = dt("tabS", [NS + 1, ELEM], BF16, "ExternalInput")
    tabA = dt("tabA", [URL_SPLIT + 1, ELEM], BF16, "ExternalInput")
    tabB = dt("tabB", [NU - URL_SPLIT + 1, ELEM], BF16, "ExternalInput")
    idxS = dt("idxS", [128, NT128 * gS * 8], mybir.dt.int16, "ExternalInput")
    idxA = dt("idxA", [128, NT128 * gA * 8], mybir.dt.int16, "ExternalInput")
    idxB = dt("idxB", [128, NT128 * gB * 8], mybir.dt.int16, "ExternalInput")
    ohS = dt("ohS", [128, NT128 * gS * 128], BF16, "ExternalInput")
    ohA = dt("ohA", [128, NT128 * gA * 128], BF16, "ExternalInput")
    ohB = dt("ohB", [128, NT128 * gB * 128], BF16, "ExternalInput")
    wbigT = dt("wbigT", [KIN, HID], BF16, "ExternalInput")
    mcomb = dt("mcomb", [12, HID], BF16, "ExternalInput")
    whead = dt("whead", [HID, 20], F32, "ExternalInput")
    outT = dt("outT", [2, NP], F32, "ExternalOutput")

    with tile.TileContext(nc) as tc:
        import contextlib
        with contextlib.ExitStack() as ctx:
            persist = ctx.enter_context(tc.tile_pool(name="persist", bufs=1))
            gpool = ctx.enter_context(tc.tile_pool(name="gath", bufs=2))
            xpool = ctx.enter_context(tc.tile_pool(name="x", bufs=2))
            ew = ctx.enter_context(tc.tile_pool(name="ew", bufs=2))
            psum = ctx.enter_context(tc.tile_pool(name="ps", bufs=2, space="PSUM"))

            # ---- persistent small tensors
            ones = persist.tile([128, 512], F32)
            nc.gpsimd.memset(ones[:], 1.0)
            gTs = persist.tile([2, NP], BF16)
            gTu = persist.tile([9, NP], BF16)
            ones2 = persist.tile([1, 512], BF16)
            nc.gpsimd.memset(ones2[:], 1.0)
            wb = persist.tile([128, NKC * HID], BF16)
            nc.sync.dma_start(
                out=wb[:].rearrange("p (c h) -> p c h", c=NKC),
                in_=wbigT.rearrange("(c p) h -> p c h", p=128))
            mcS = persist.tile([2, HID], BF16)
            nc.sync.dma_start(out=mcS[:], in_=mcomb[0:2, :])
            mcU = persist.tile([9, HID], BF16)
            nc.sync.dma_start(out=mcU[:], in_=mcomb[2:11, :])
            mcC = persist.tile([1, HID], BF16)
            nc.sync.dma_start(out=mcC[:], in_=mcomb[11:12, :])
            wh = persist.tile([HID, 20], F32)
            nc.sync.dma_start(out=wh[:], in_=whead[:])

            # ---- phase B emitter (interleaved with phase A chunks)
            def phase_b(nt):
                ns = slice(nt * 512, (nt + 1) * 512)
                xs = xpool.tile([128, NKC * 512], BF16, tag="xs")
                nc.sync.dma_start(
                    out=xs[:].rearrange("p (c n) -> p c n", c=NKC),
                    in_=xT[:, ns].rearrange("(c p) n -> p c n", p=128))
                pP = psum.tile([128, 512], F32, space="PSUM", tag="pP")
                for k in range(NKC):
                    nc.tensor.matmul(
                        out=pP[:], lhsT=wb[:, k * HID:(k + 1) * HID],
                        rhs=xs[:, k * 512:(k + 1) * 512],
                        start=(k == 0), stop=False)
                nc.tensor.matmul(out=pP[:], lhsT=mcS[:], rhs=gTs[:, ns],
                                 start=False, stop=False)
                nc.tensor.matmul(out=pP[:], lhsT=mcU[:], rhs=gTu[:, ns],
                                 start=False, stop=False)
                nc.tensor.matmul(out=pP[:], lhsT=mcC[:], rhs=ones2[:],
                                 start=False, stop=True)

                xt = ew.tile([128, 512], F32, tag="xt")      # clamp(h,0,2.2)
                nc.vector.tensor_scalar(out=xt[:], in0=pP[:], scalar1=0.0,
                                        scalar2=XCLAMP,
                                        op0=mybir.AluOpType.max,
                                        op1=mybir.AluOpType.min)
                sil = ew.tile([128, 512], F32, tag="sil")
                nc.scalar.activation(sil[:], pP[:],
                                     mybir.ActivationFunctionType.Silu)
                rsil = ew.tile([128, 512], F32, tag="rsil")
                nc.vector.tensor_scalar_max(rsil[:], sil[:], 0.0)
                x2 = ew.tile([128, 512], F32, tag="x2")
                nc.scalar.square(x2[:], xt[:])
                x3 = ew.tile([128, 512], F32, tag="x3")
                nc.vector.tensor_tensor(out=x3[:], in0=x2[:], in1=xt[:],
                                        op=mybir.AluOpType.mult)
                r3s = []
                for k, tk in enumerate(KNOTS):
                    r = ew.tile([128, 512], F32, tag="r")
                    nc.vector.tensor_scalar(out=r[:], in0=xt[:], scalar1=tk,
                                            scalar2=0.0,
                                            op0=mybir.AluOpType.subtract,
                                            op1=mybir.AluOpType.max)
                    r2 = ew.tile([128, 512], F32, tag="rr")
                    nc.scalar.square(r2[:], r[:])
                    r3 = ew.tile([128, 512], F32, tag=f"rrr{k}")
                    nc.vector.tensor_tensor(out=r3[:], in0=r2[:], in1=r[:],
                                            op=mybir.AluOpType.mult)
                    r3s.append(r3)
                pO = psum.tile([2, 512], F32, space="PSUM", tag="pO")
                chunks = [rsil, ones, xt, x2, x3] + r3s
                for j, ck in enumerate(chunks):
                    nc.tensor.matmul(out=pO[:], lhsT=wh[:, 2 * j:2 * j + 2],
                                     rhs=ck[:], start=(j == 0), stop=(j == 9))
                ot = ew.tile([2, 512], F32, tag="ot")
                nc.vector.tensor_copy(out=ot[:], in_=pO[:])
                nc.sync.dma_start(out=outT[:, ns], in_=ot[:])

            # ---- phase A: gather + one-hot scatter matmuls
            next_nt = [0]
            for ch in range(N_CH):
                t0 = ch * CH_T
                cls = []
                for tab, idxd, g, ohd in ((tabS, idxS, gS, ohS),
                                          (tabA, idxA, gA, ohA),
                                          (tabB, idxB, gB, ohB)):
                    nidx = CH_T * g * 128
                    isb = gpool.tile([128, nidx // 16], mybir.dt.int16,
                                     tag=f"i{g}_{id(idxd)}")
                    nc.sync.dma_start(
                        out=isb[:], in_=idxd[:, t0 * g * 8:(t0 + CH_T) * g * 8])
                    gt = gpool.tile([128, CH_T * g, ELEM], BF16,
                                    tag=f"g{id(idxd)}")
                    nc.gpsimd.dma_gather(
                        out_ap=gt[:], in_ap=tab[:], idxs_ap=isb[:],
                        num_idxs=nidx, num_idxs_reg=nidx, elem_size=ELEM,
                        single_packet=False)
                    ohsb = gpool.tile([128, CH_T * g * 128], BF16,
                                      tag=f"oh{id(idxd)}")
                    nc.sync.dma_start(
                        out=ohsb[:],
                        in_=ohd[:, t0 * g * 128:(t0 + CH_T) * g * 128])
                    cls.append((gt, g, ohsb))
                for tl in range(CH_T):
                    t = t0 + tl
                    pS = psum.tile([2, 128], F32, space="PSUM", tag="pS")
                    pU = psum.tile([9, 128], F32, space="PSUM", tag="pU")
                    for ci, (gt, g, ohsb) in enumerate(cls):
                        for gi in range(g):
                            gcol = tl * g + gi
                            oh = ohsb[:, gcol * 128:(gcol + 1) * 128]
                            lhs = gt[:, gcol, 0:2 if ci == 0 else 9]
                            if ci == 0:
                                nc.tensor.matmul(out=pS[:], lhsT=lhs, rhs=oh,
                                                 start=(gi == 0),
                                                 stop=(gi == g - 1))
                            else:
                                nc.tensor.matmul(out=pU[:], lhsT=lhs, rhs=oh,
                                                 start=(ci == 1 and gi == 0),
                                                 stop=(ci == 2 and gi == g - 1))
                    sl = slice(t * 128, (t + 1) * 128)
                    nc.scalar.copy(out=gTs[:, sl], in_=pS[:])
                    nc.scalar.copy(out=gTu[:, sl], in_=pU[:])
                ready = ((ch + 1) * CH_T) // 4          # node tiles with gT done
                while next_nt[0] < (ready if ch < N_CH - 1 else NT512):
                    phase_b(next_nt[0])
                    next_nt[0] += 1


    nc.compile()
    return nc


# ----------------------------------------------------------------- entry point
def kernel(**inp):
    inp = {k: np.asarray(v) for k, v in inp.items()}
    wbigT, mcomb, head = _fold_weights(inp)
    eprep = _prep_edges(inp)
    (idxS, dstS, gS), (idxA, dstA, gA), (idxB, dstB, gB) = eprep

    key = (gS, gA, gB)
    if key not in _CACHE:
        _CACHE[key] = _build(gS, gA, gB)
    nc = _CACHE[key]

    tabS = np.zeros((NS + 1, ELEM), BF)
    tabS[:NS, 0] = inp["x_sender"][:, 0].astype(BF)
    tabS[:NS, 1] = 1
    tabA = np.zeros((URL_SPLIT + 1, ELEM), BF)
    tabA[:URL_SPLIT, 0:8] = inp["x_url"][:URL_SPLIT].astype(BF)
    tabA[:URL_SPLIT, 8] = 1
    tabB = np.zeros((NU - URL_SPLIT + 1, ELEM), BF)
    tabB[: NU - URL_SPLIT, 0:8] = inp["x_url"][URL_SPLIT:].astype(BF)
    tabB[: NU - URL_SPLIT, 8] = 1

    in_maps = []
    for c in range(N_CORES):
        xsh = np.zeros((KIN, NP), BF)
        xsh[:, :NSH] = inp["x_email"][c * NSH:(c + 1) * NSH].T.astype(BF)
        in_maps.append({
            "xT": xsh, "tabS": tabS, "tabA": tabA, "tabB": tabB,
            "idxS": idxS[c], "idxA": idxA[c], "idxB": idxB[c],
            "ohS": dstS[c], "ohA": dstA[c], "ohB": dstB[c],
            "wbigT": wbigT.astype(BF), "mcomb": mcomb.astype(BF), "whead": head,
        })

    global _LAST_RESULT
    trace = os.environ.get("KERNEL_TRACE", "0") == "1"
    res = run_bass_kernel_spmd(nc, in_maps, core_ids=list(range(N_CORES)),
                               trace=trace)
    _LAST_RESULT = res
    out = np.empty((NE, 2), np.float32)
    for c in range(N_CORES):
        out[c * NSH:(c + 1) * NSH] = res.results[c]["outT"][:, :NSH].T
    return out



# revision 26
# speedup vs baseline: 4.2457x; 4.2457x over previous
"""Trainium2 Bass kernel for nn_HKANGNN (hetero GraphConv + KAN head).

Math (only the email-node output path matters):
  e    = x_email @ w_email.T + b_email
  agg_se[n] = sum_{se edges -> n} (x_sender[src] @ w_sender.T + b_sender)
  agg_ue[n] = sum_{ue edges -> n} (x_url[src]    @ w_url.T    + b_url)
  out_e = agg_se @ w_rel_se.T + b_rel_se + agg_ue @ w_rel_ue.T + b_rel_ue
        + e @ (w_root_se + w_root_ue).T
  h = relu(out_e);  out = silu(h) @ base_w.T + einsum(b_splines(h), spline_w)

Device strategy (8 cores, email nodes sharded 12500/core, padded to 12800):
  * linearity folds the tiny projections into an 11-feature per-edge payload
    [s_val, s_cnt, u0..u7, u_cnt]; host pre-gathers payload rows per edge
    (no on-device gather) and prebuilds the dst one-hot blocks.
  * segment-sum = one matmul per 128-edge group: stationary payload
    [128, 11] x one-hot [128, 128] accumulated into an [11, 512] PSUM
    window (4 dst tiles), copied to a persistent gT [12, NP] (row 11 = 1).
  * projection: out_e.T accumulated over 6 K-chunks of (Wrootsum@w_email).T
    plus one [12]-contract matmul with mc (rel weights + bias) on gT.
  * KAN head (all bf16): h>=0 and all bases vanish for h>=2.2, so with
    x = clamp(h,0,2.2):  spline(h) = q0 + q1 x + q2 x^2 + q3 x^3
      + sum_k W'_k relu(x-t_k)^3   (t_k = .2,.6,1.0,1.4,1.8)
    -> 9 bf16 matmul chunks [silu,x,x^2,x^3,R1^3..R5^3] into [2,512] PSUM,
    q0 folded into the PSUM->SBUF copy (per-partition bias add).
"""

import os
import numpy as np
import ml_dtypes

import concourse.bass as bass
import concourse.mybir as mybir
import concourse.tile as tile
from concourse import bacc
from concourse.bass_utils import run_bass_kernel_spmd

F32 = mybir.dt.float32
BF16 = mybir.dt.bfloat16
BF = ml_dtypes.bfloat16

N_CORES = 8
HID = 128
NE, NS, NU = 100000, 30000, 50000
NSH = NE // N_CORES          # 12500 real nodes per core
NP = 12800                   # padded (25 x 512 node tiles, 100 x 128 dst tiles)
NT128 = NP // 128            # 100 dst tiles
NT512 = NP // 512            # 25 node tiles
KIN = 768
NKC = KIN // 128             # 6 projection K-chunks
NF = 11                      # payload features [s_val, s_cnt, u0..u7, u_cnt]
FPAD = 16                    # payload row padded to 16 bf16
CH_T = 10                    # dst tiles per stream chunk
N_CH = NT128 // CH_T         # 10 chunks
KNOTS = (0.2, 0.6, 1.0, 1.4, 1.8)
MIRROR = (0, 1, 2)           # knots evaluated as relu(t-x)^3, cube folded to poly
XCLAMP = 2.2
FP16 = mybir.dt.float16
F32R = mybir.dt.float32r

_LAST_RESULT = None
_CACHE = {}


# ----------------------------------------------------------------- host folds
def _head_weights(base_w, spline_w):
    """[128, 20] f32: lhsT ([d,2]) per head chunk, order
    [silu, ones, x, x^2, x^3, R(.2)^3, R(.6)^3, R(1.0)^3, R(1.4)^3, R(1.8)^3]."""
    c = np.array([1.0, -4.0, 6.0, -4.0, 1.0], np.float64)
    h = 0.4
    scale = 1.0 / (6.0 * h ** 3)
    O, D, B = spline_w.shape                      # [2, 128, 8]
    wp = np.zeros((O, D, 11), np.float64)         # W'[o,d,m], m=0..10
    for m in range(11):
        for j in range(5):
            b = m - j
            if 0 <= b < B:
                wp[:, :, m] += spline_w[:, :, b].astype(np.float64) * c[j] * scale
    t = np.arange(11) * h - 2.2                   # knot m at t_m
    q = np.zeros((4, O, D), np.float64)           # poly coeffs from m=0..5
    for m in range(6):
        q[0] += -t[m] ** 3 * wp[:, :, m]
        q[1] += 3 * t[m] ** 2 * wp[:, :, m]
        q[2] += -3 * t[m] * wp[:, :, m]
        q[3] += wp[:, :, m]
    head = np.zeros((D, 20), np.float64)
    head[:, 0:2] = base_w.T                       # silu chunk
    for j in range(4):                            # ones, x, x^2, x^3
        head[:, 2 * (1 + j):2 * (1 + j) + 2] = q[j].T
    for k in range(5):                            # relu^3 knots m=6..10
        head[:, 2 * (5 + k):2 * (5 + k) + 2] = wp[:, :, 6 + k].T
    return head.astype(np.float32)


def _fold_weights(inp):
    wrs = inp["w_root_se"] + inp["w_root_ue"]
    wbigT = (wrs @ inp["w_email"]).T.copy()                     # [768, 128]
    mcomb = np.zeros((12, 128), np.float32)
    mcomb[0] = inp["w_rel_se"] @ inp["w_sender"][:, 0]
    mcomb[1] = inp["w_rel_se"] @ inp["b_sender"]
    mcomb[2:10] = (inp["w_rel_ue"] @ inp["w_url"]).T
    mcomb[10] = inp["w_rel_ue"] @ inp["b_url"]
    mcomb[11] = inp["b_rel_se"] + inp["b_rel_ue"] + wrs @ inp["b_email"]
    head = _head_weights(inp["base_w"], inp["spline_w"])
    return wbigT, mcomb, head


def _prep_edges(inp):
    """Merged-class per-core payload + one-hot arrays.

    Returns (gts, pays, ohs): gts = per-dst-tile group counts (shared across
    cores); pays[c] = [128, G, FPAD] bf16; ohs[c] = [128, G*128] bf16."""
    se_src, se_dst = inp["se_src"], inp["se_dst"]
    ue_src, ue_dst = inp["ue_src"], inp["ue_dst"]
    xs = inp["x_sender"][:, 0].astype(np.float32)
    xu = inp["x_url"].astype(np.float32)

    counts = np.zeros((N_CORES, NT128), np.int64)
    percore = []
    for c in range(N_CORES):
        off = c * NSH
        ms = (se_dst >= off) & (se_dst < off + NSH)
        mu = (ue_dst >= off) & (ue_dst < off + NSH)
        dse = se_dst[ms] - off
        due = ue_dst[mu] - off
        dloc = np.concatenate([dse, due])
        n_se = len(dse)
        payload = np.zeros((len(dloc), FPAD), np.float32)
        payload[:n_se, 0] = xs[se_src[ms]]
        payload[:n_se, 1] = 1.0
        payload[n_se:, 2:10] = xu[ue_src[mu]]
        payload[n_se:, 10] = 1.0
        counts[c] = np.bincount(dloc >> 7, minlength=NT128)
        percore.append((dloc, payload))

    gts = tuple(max(1, int(-(-m // 128))) for m in counts.max(axis=0))
    B = np.concatenate([[0], np.cumsum(gts)]).astype(np.int64)  # group bases
    G = int(B[-1])

    pays, ohs = [], []
    for c in range(N_CORES):
        dloc, payload = percore[c]
        n = len(dloc)
        tiles = dloc >> 7
        order = np.argsort(tiles, kind="stable")
        st = tiles[order]
        starts = np.searchsorted(st, np.arange(NT128))
        rank = np.arange(n) - starts[st]
        grp = B[st] + (rank >> 7)
        part = rank & 127
        pay = np.zeros((128, G, FPAD), BF)
        pay[part, grp, :] = payload[order].astype(BF)
        oh = np.zeros((128, G * 128), BF)
        oh[part, grp * 128 + (dloc[order] & 127)] = 1.0
        pays.append(pay)
        ohs.append(oh)
    return gts, pays, ohs


# ----------------------------------------------------------------- device build
def _build(gts):
    B = [0]
    for g in gts:
        B.append(B[-1] + g)
    G = B[-1]

    nc = bacc.Bacc("TRN2", target_bir_lowering=False, debug=False,
                   num_devices=N_CORES)
    dt = lambda n, s, d, k: nc.dram_tensor(n, s, d, kind=k).ap()
    xT = dt("xT", [KIN, NP], BF16, "ExternalInput")
    pay = dt("pay", [128, G * FPAD], BF16, "ExternalInput")
    oh = dt("oh", [128, G * 128], BF16, "ExternalInput")
    wbigT = dt("wbigT", [KIN, HID], BF16, "ExternalInput")
    mc = dt("mc", [12, HID], BF16, "ExternalInput")
    whb = dt("whb", [HID, 12], FP16, "ExternalInput")
    wh3 = dt("wh3", [HID, 6], F32, "ExternalInput")
    q0 = dt("q0", [2, 1], F32, "ExternalInput")
    outT = dt("outT", [2, NP], F32, "ExternalOutput")

    AF = mybir.ActivationFunctionType
    OP = mybir.AluOpType

    with tile.TileContext(nc) as tc:
        import contextlib
        with contextlib.ExitStack() as ctx:
            persist = ctx.enter_context(tc.tile_pool(name="persist", bufs=1))
            dpool = ctx.enter_context(tc.tile_pool(name="edges", bufs=2))
            xpool = ctx.enter_context(tc.tile_pool(name="x", bufs=2))
            ew = ctx.enter_context(tc.tile_pool(name="ew", bufs=2))
            psA = ctx.enter_context(tc.tile_pool(name="psA", bufs=2, space="PSUM"))
            psB = ctx.enter_context(tc.tile_pool(name="psB", bufs=2, space="PSUM"))
            psO = ctx.enter_context(tc.tile_pool(name="psO", bufs=2, space="PSUM"))

            # ---- persistent small tensors
            wb = persist.tile([128, NKC * HID], BF16)
            nc.sync.dma_start(
                out=wb[:].rearrange("p (c h) -> p c h", c=NKC),
                in_=wbigT.rearrange("(c p) h -> p c h", p=128))
            mcS = persist.tile([11, HID], BF16)
            nc.sync.dma_start(out=mcS[:], in_=mc[0:11, :])
            mcC = persist.tile([1, HID], BF16)
            nc.sync.dma_start(out=mcC[:], in_=mc[11:12, :])
            ones2 = persist.tile([1, 512], BF16)
            nc.gpsimd.memset(ones2[:], 1.0)
            whB = persist.tile([HID, 12], FP16)
            nc.sync.dma_start(out=whB[:], in_=whb[:])
            wh32 = persist.tile([HID, 6], F32)
            nc.sync.dma_start(out=wh32[:], in_=wh3[:])
            whR = persist.tile([HID, 6], F32R)
            nc.vector.tensor_copy(out=whR[:], in_=wh32[:])
            scm1 = persist.tile([128, 1], F32)
            nc.gpsimd.memset(scm1[:], -1.0)
            q0S = persist.tile([2, 1], F32)
            nc.sync.dma_start(out=q0S[:], in_=q0[:])
            gT = persist.tile([NF, NP], BF16)
            knotb = persist.tile([128, len(KNOTS)], F32)
            for k, tk in enumerate(KNOTS):
                nc.gpsimd.memset(knotb[:, k:k + 1],
                                 tk if k in MIRROR else -tk)

            # ---- phase B emitter (interleaved with phase A chunks)
            def phase_b(nt):
                ns = slice(nt * 512, (nt + 1) * 512)
                xs = xpool.tile([128, NKC * 512], BF16, tag="xs")
                nc.sync.dma_start(
                    out=xs[:].rearrange("p (c n) -> p c n", c=NKC),
                    in_=xT[:, ns].rearrange("(c p) n -> p c n", p=128))
                pP = psB.tile([128, 512], F32, space="PSUM", tag="pP")
                for k in range(NKC):
                    nc.tensor.matmul(
                        out=pP[:], lhsT=wb[:, k * HID:(k + 1) * HID],
                        rhs=xs[:, k * 512:(k + 1) * 512],
                        start=(k == 0), stop=False)
                nc.tensor.matmul(out=pP[:], lhsT=mcS[:], rhs=gT[:, ns],
                                 start=False, stop=False)
                nc.tensor.matmul(out=pP[:], lhsT=mcC[:], rhs=ones2[:],
                                 start=False, stop=True)

                # h = relu(pP); x = min(h, 2.2).  The folded cubic coeffs are
                # large and cancelling, so the poly chunks [x, x^2, x^3] stay
                # f32 (fp32r matmuls); base + knot cubes are bf16-safe.
                xt = ew.tile([128, 512], F32R, tag="xt")
                nc.vector.tensor_scalar(out=xt[:], in0=pP[:], scalar1=0.0,
                                        scalar2=XCLAMP,
                                        op0=OP.max, op1=OP.min)
                sil = ew.tile([128, 512], FP16, tag="sil")
                nc.scalar.activation(sil[:], pP[:], AF.Silu)
                rsl = ew.tile([128, 512], FP16, tag="rsl")
                nc.vector.tensor_scalar_max(rsl[:], sil[:], 0.0)
                x2 = ew.tile([128, 512], F32R, tag="x2")
                nc.scalar.square(x2[:], xt[:])
                x3 = ew.tile([128, 512], F32R, tag="x3")
                nc.vector.scalar_tensor_tensor(
                    out=x3[:], in0=x2[:], scalar=0.0, in1=xt[:],
                    op0=OP.bypass, op1=OP.mult)
                r3s = []
                for k, tk in enumerate(KNOTS):
                    rk = ew.tile([128, 512], FP16, tag="rk")
                    nc.scalar.activation(
                        rk[:], xt[:], AF.Relu, bias=knotb[:, k:k + 1],
                        scale=scm1[:, 0:1] if k in MIRROR else 1.0)
                    r2 = ew.tile([128, 512], FP16, tag="r2")
                    nc.vector.tensor_tensor(out=r2[:], in0=rk[:], in1=rk[:],
                                            op=OP.mult)
                    r3 = ew.tile([128, 512], FP16, tag=f"r3{k}")
                    nc.vector.tensor_tensor(out=r3[:], in0=r2[:], in1=rk[:],
                                            op=OP.mult)
                    r3s.append(r3)
                pO = psO.tile([2, 512], F32, space="PSUM", tag="pO")
                for j, ck in enumerate([xt, x2, x3]):
                    nc.tensor.matmul(out=pO[:],
                                     lhsT=whR[:, 2 * j:2 * j + 2],
                                     rhs=ck[:],
                                     start=(j == 0), stop=False)
                for j, ck in enumerate([rsl] + r3s):
                    nc.tensor.matmul(out=pO[:], lhsT=whB[:, 2 * j:2 * j + 2],
                                     rhs=ck[:], start=False, stop=(j == 5))
                ot = ew.tile([2, 512], F32, tag="ot")
                nc.scalar.activation(ot[:], pO[:], AF.Identity, bias=q0S[:])
                nc.sync.dma_start(out=outT[:, ns], in_=ot[:])

            # ---- phase A: payload x one-hot scatter matmuls
            next_nt = [0]
            pS = [None]
            for ch in range(N_CH):
                t0 = ch * CH_T
                g0, g1 = B[t0], B[t0 + CH_T]
                cg = g1 - g0
                psb = dpool.tile([128, cg, FPAD], BF16, tag="pay")
                nc.sync.dma_start(
                    out=psb[:], in_=pay[:, g0 * FPAD:g1 * FPAD]
                    .rearrange("p (g f) -> p g f", f=FPAD))
                ohsb = dpool.tile([128, cg * 128], BF16, tag="oh")
                nc.sync.dma_start(out=ohsb[:], in_=oh[:, g0 * 128:g1 * 128])
                for t in range(t0, t0 + CH_T):
                    w = t % 4
                    if w == 0:
                        pS[0] = psA.tile([NF, 512], F32, space="PSUM",
                                         tag="pS", name="pS")
                    for gi in range(gts[t]):
                        gg = B[t] + gi - g0
                        nc.tensor.matmul(
                            out=pS[0][:, w * 128:(w + 1) * 128],
                            lhsT=psb[:, gg, 0:NF],
                            rhs=ohsb[:, gg * 128:(gg + 1) * 128],
                            start=(gi == 0), stop=(gi == gts[t] - 1))
                    if w == 3:
                        t4 = t // 4
                        nc.scalar.copy(
                            out=gT[:, t4 * 512:(t4 + 1) * 512], in_=pS[0][:])
                ready = ((ch + 1) * CH_T) // 4
                while next_nt[0] < (ready if ch < N_CH - 1 else NT512):
                    phase_b(next_nt[0])
                    next_nt[0] += 1

    nc.compile()
    return nc


# ----------------------------------------------------------------- entry point
def kernel(**inp):
    inp = {k: np.asarray(v) for k, v in inp.items()}
    wbigT, mcomb, head = _fold_weights(inp)
    gts, pays, ohs = _prep_edges(inp)

    if gts not in _CACHE:
        _CACHE[gts] = _build(gts)
    nc = _CACHE[gts]

    # fp16 chunks: [silu, knot cubes (mirrored for k in MIRROR)];
    # f32 chunks: [x, x^2, x^3] + q0 bias, with mirrored knots' full cubes
    # folded into the poly: w*relu(x-t)^3 = w*(x-t)^3 + w*relu(t-x)^3.
    qf = head[:, 2:10].astype(np.float64).copy()        # [128, 8] q0..q3 pairs
    for k in MIRROR:
        t = KNOTS[k]
        w = head[:, 10 + 2 * k:12 + 2 * k].astype(np.float64)
        qf[:, 0:2] += -t ** 3 * w
        qf[:, 2:4] += 3 * t ** 2 * w
        qf[:, 4:6] += -3 * t * w
        qf[:, 6:8] += w
    whb_np = np.ascontiguousarray(
        np.concatenate([head[:, 0:2], head[:, 10:20]], axis=1)
    ).astype(np.float16)
    wh3_np = np.ascontiguousarray(qf[:, 2:8]).astype(np.float32)
    q0v = qf[:, 0:2].sum(axis=0).astype(np.float32).reshape(2, 1)

    in_maps = []
    for c in range(N_CORES):
        xsh = np.zeros((KIN, NP), BF)
        xsh[:, :NSH] = inp["x_email"][c * NSH:(c + 1) * NSH].T.astype(BF)
        in_maps.append({
            "xT": xsh,
            "pay": pays[c].reshape(128, -1),
            "oh": ohs[c],
            "wbigT": wbigT.astype(BF), "mc": mcomb.astype(BF),
            "whb": whb_np, "wh3": wh3_np, "q0": q0v,
        })

    global _LAST_RESULT
    trace = os.environ.get("KERNEL_TRACE", "0") == "1"
    res = run_bass_kernel_spmd(nc, in_maps, core_ids=list(range(N_CORES)),
                               trace=trace)
    _LAST_RESULT = res
    out = np.empty((NE, 2), np.float32)
    for c in range(N_CORES):
        out[c * NSH:(c + 1) * NSH] = res.results[c]["outT"][:, :NSH].T
    return out


# revision 37
# speedup vs baseline: 4.4694x; 1.0527x over previous
"""Trainium2 Bass kernel for nn_HKANGNN (hetero GraphConv + KAN head).

Math (only the email-node output path matters):
  e    = x_email @ w_email.T + b_email
  agg_se[n] = sum_{se edges -> n} (x_sender[src] @ w_sender.T + b_sender)
  agg_ue[n] = sum_{ue edges -> n} (x_url[src]    @ w_url.T    + b_url)
  out_e = agg_se @ w_rel_se.T + b_rel_se + agg_ue @ w_rel_ue.T + b_rel_ue
        + e @ (w_root_se + w_root_ue).T
  h = relu(out_e);  out = silu(h) @ base_w.T + einsum(b_splines(h), spline_w)

Device strategy (8 cores, email nodes sharded 12500/core, padded to 12800):
  * linearity folds the tiny projections into an 11-feature per-edge payload
    [s_val, s_cnt, u0..u7, u_cnt]; host pre-gathers payload rows per edge
    (no on-device gather) and prebuilds the dst one-hot blocks.
  * segment-sum = one matmul per 128-edge group: stationary payload
    [128, 11] x one-hot [128, 128] accumulated into an [11, 512] PSUM
    window (4 dst tiles), copied to a persistent gT [12, NP] (row 11 = 1).
  * projection: out_e.T accumulated over 6 K-chunks of (Wrootsum@w_email).T
    plus one [12]-contract matmul with mc (rel weights + bias) on gT.
  * KAN head (all bf16): h>=0 and all bases vanish for h>=2.2, so with
    x = clamp(h,0,2.2):  spline(h) = q0 + q1 x + q2 x^2 + q3 x^3
      + sum_k W'_k relu(x-t_k)^3   (t_k = .2,.6,1.0,1.4,1.8)
    -> 9 bf16 matmul chunks [silu,x,x^2,x^3,R1^3..R5^3] into [2,512] PSUM,
    q0 folded into the PSUM->SBUF copy (per-partition bias add).
"""

import os
import numpy as np
import ml_dtypes

import concourse.bass as bass
import concourse.mybir as mybir
import concourse.tile as tile
from concourse import bacc
from concourse.bass_utils import run_bass_kernel_spmd

F32 = mybir.dt.float32
BF16 = mybir.dt.bfloat16
BF = ml_dtypes.bfloat16

N_CORES = 8
HID = 128
NE, NS, NU = 100000, 30000, 50000
NSH = NE // N_CORES          # 12500 real nodes per core
NP = 12800                   # padded (25 x 512 node tiles, 100 x 128 dst tiles)
NT128 = NP // 128            # 100 dst tiles
NT512 = NP // 512            # 25 node tiles
KIN = 768
NKC = KIN // 128             # 6 projection K-chunks
NF = 11                      # payload features [s_val, s_cnt, u0..u7, u_cnt]
FPAD = 16                    # payload row padded to 16 bf16
CH_T = 10                    # dst tiles per stream chunk
N_CH = NT128 // CH_T         # 10 chunks
KNOTS = (0.2, 0.6, 1.0, 1.4, 1.8)
MIRROR = (0, 1, 2)           # knots evaluated as relu(t-x)^3, cube folded to poly
XCLAMP = 2.2
FP16 = mybir.dt.float16
F32R = mybir.dt.float32r

_LAST_RESULT = None
_CACHE = {}


# ----------------------------------------------------------------- host folds
def _head_weights(base_w, spline_w):
    """[128, 20] f32: lhsT ([d,2]) per head chunk, order
    [silu, ones, x, x^2, x^3, R(.2)^3, R(.6)^3, R(1.0)^3, R(1.4)^3, R(1.8)^3]."""
    c = np.array([1.0, -4.0, 6.0, -4.0, 1.0], np.float64)
    h = 0.4
    scale = 1.0 / (6.0 * h ** 3)
    O, D, B = spline_w.shape                      # [2, 128, 8]
    wp = np.zeros((O, D, 11), np.float64)         # W'[o,d,m], m=0..10
    for m in range(11):
        for j in range(5):
            b = m - j
            if 0 <= b < B:
                wp[:, :, m] += spline_w[:, :, b].astype(np.float64) * c[j] * scale
    t = np.arange(11) * h - 2.2                   # knot m at t_m
    q = np.zeros((4, O, D), np.float64)           # poly coeffs from m=0..5
    for m in range(6):
        q[0] += -t[m] ** 3 * wp[:, :, m]
        q[1] += 3 * t[m] ** 2 * wp[:, :, m]
        q[2] += -3 * t[m] * wp[:, :, m]
        q[3] += wp[:, :, m]
    head = np.zeros((D, 20), np.float64)
    head[:, 0:2] = base_w.T                       # silu chunk
    for j in range(4):                            # ones, x, x^2, x^3
        head[:, 2 * (1 + j):2 * (1 + j) + 2] = q[j].T
    for k in range(5):                            # relu^3 knots m=6..10
        head[:, 2 * (5 + k):2 * (5 + k) + 2] = wp[:, :, 6 + k].T
    return head.astype(np.float32)


def _fold_weights(inp):
    wrs = inp["w_root_se"] + inp["w_root_ue"]
    wbigT = (wrs @ inp["w_email"]).T.copy()                     # [768, 128]
    mcomb = np.zeros((12, 128), np.float32)
    mcomb[0] = inp["w_rel_se"] @ inp["w_sender"][:, 0]
    mcomb[1] = inp["w_rel_se"] @ inp["b_sender"]
    mcomb[2:10] = (inp["w_rel_ue"] @ inp["w_url"]).T
    mcomb[10] = inp["w_rel_ue"] @ inp["b_url"]
    mcomb[11] = inp["b_rel_se"] + inp["b_rel_ue"] + wrs @ inp["b_email"]
    head = _head_weights(inp["base_w"], inp["spline_w"])
    return wbigT, mcomb, head


def _prep_edges(inp):
    """Merged-class per-core payload + one-hot arrays.

    Returns (gts, pays, ohs): gts = per-dst-tile group counts (shared across
    cores); pays[c] = [128, G, FPAD] bf16; ohs[c] = [128, G*128] bf16."""
    se_src, se_dst = inp["se_src"], inp["se_dst"]
    ue_src, ue_dst = inp["ue_src"], inp["ue_dst"]
    xs = inp["x_sender"][:, 0].astype(np.float32)
    xu = inp["x_url"].astype(np.float32)

    counts = np.zeros((N_CORES, NT128), np.int64)
    percore = []
    for c in range(N_CORES):
        off = c * NSH
        ms = (se_dst >= off) & (se_dst < off + NSH)
        mu = (ue_dst >= off) & (ue_dst < off + NSH)
        dse = se_dst[ms] - off
        due = ue_dst[mu] - off
        dloc = np.concatenate([dse, due])
        n_se = len(dse)
        payload = np.zeros((len(dloc), FPAD), np.float32)
        payload[:n_se, 0] = xs[se_src[ms]]
        payload[:n_se, 1] = 1.0
        payload[n_se:, 2:10] = xu[ue_src[mu]]
        payload[n_se:, 10] = 1.0
        counts[c] = np.bincount(dloc >> 7, minlength=NT128)
        percore.append((dloc, payload))

    gts = tuple(max(1, int(-(-m // 128))) for m in counts.max(axis=0))
    B = np.concatenate([[0], np.cumsum(gts)]).astype(np.int64)  # group bases
    G = int(B[-1])

    pays, ohs = [], []
    for c in range(N_CORES):
        dloc, payload = percore[c]
        n = len(dloc)
        tiles = dloc >> 7
        order = np.argsort(tiles, kind="stable")
        st = tiles[order]
        starts = np.searchsorted(st, np.arange(NT128))
        rank = np.arange(n) - starts[st]
        grp = B[st] + (rank >> 7)
        part = rank & 127
        pay = np.zeros((128, G, FPAD), BF)
        pay[part, grp, :] = payload[order].astype(BF)
        oh = np.zeros((128, G * 128), BF)
        oh[part, grp * 128 + (dloc[order] & 127)] = 1.0
        pays.append(pay)
        ohs.append(oh)
    return gts, pays, ohs


# ----------------------------------------------------------------- device build
def _build(gts):
    B = [0]
    for g in gts:
        B.append(B[-1] + g)
    G = B[-1]

    nc = bacc.Bacc("TRN2", target_bir_lowering=False, debug=False,
                   num_devices=N_CORES)
    dt = lambda n, s, d, k: nc.dram_tensor(n, s, d, kind=k).ap()
    xT = dt("xT", [KIN, NP], BF16, "ExternalInput")
    pay = dt("pay", [128, G * FPAD], BF16, "ExternalInput")
    oh = dt("oh", [128, G * 128], BF16, "ExternalInput")
    wbigT = dt("wbigT", [KIN, HID], BF16, "ExternalInput")
    mc = dt("mc", [12, HID], BF16, "ExternalInput")
    whb = dt("whb", [HID, 12], FP16, "ExternalInput")
    wh3 = dt("wh3", [HID, 6], F32, "ExternalInput")
    q0 = dt("q0", [2, 1], F32, "ExternalInput")
    outT = dt("outT", [2, NP], F32, "ExternalOutput")

    AF = mybir.ActivationFunctionType
    OP = mybir.AluOpType

    with tile.TileContext(nc) as tc:
        import contextlib
        with contextlib.ExitStack() as ctx:
            persist = ctx.enter_context(tc.tile_pool(name="persist", bufs=1))
            dpool = ctx.enter_context(tc.tile_pool(name="edges", bufs=2))
            xpool = ctx.enter_context(tc.tile_pool(name="x", bufs=2))
            ew = ctx.enter_context(tc.tile_pool(name="ew", bufs=2))
            psA = ctx.enter_context(tc.tile_pool(name="psA", bufs=2, space="PSUM"))
            psB = ctx.enter_context(tc.tile_pool(name="psB", bufs=2, space="PSUM"))
            psO = ctx.enter_context(tc.tile_pool(name="psO", bufs=1, space="PSUM"))

            # ---- persistent small tensors
            wb = persist.tile([128, NKC * HID], BF16)
            nc.sync.dma_start(
                out=wb[:].rearrange("p (c h) -> p c h", c=NKC),
                in_=wbigT.rearrange("(c p) h -> p c h", p=128))
            mcS = persist.tile([12, HID], BF16)
            nc.sync.dma_start(out=mcS[:], in_=mc[:])
            whB = persist.tile([HID, 12], FP16)
            nc.sync.dma_start(out=whB[:], in_=whb[:])
            wh32 = persist.tile([HID, 6], F32)
            nc.sync.dma_start(out=wh32[:], in_=wh3[:])
            whR = persist.tile([HID, 6], F32R)
            nc.vector.tensor_copy(out=whR[:], in_=wh32[:])
            scm1 = persist.tile([128, 1], F32)
            nc.gpsimd.memset(scm1[:], -1.0)
            q0S = persist.tile([2, 1], F32)
            nc.sync.dma_start(out=q0S[:], in_=q0[:])
            # rows 0:11 overwritten per window by phase A; row 11 stays 1.0
            # (bias row for the mc matmul)
            gT = persist.tile([12, NP], BF16)
            nc.gpsimd.memset(gT[:], 1.0)
            knotb = persist.tile([128, len(KNOTS)], F32)
            for k, tk in enumerate(KNOTS):
                nc.gpsimd.memset(knotb[:, k:k + 1],
                                 tk if k in MIRROR else -tk)

            # ---- phase B emitter (1024 nodes per call, interleaved with A)
            NB = 1024
            BLOCKS = [(i * NB, min(NB, NP - i * NB))
                      for i in range((NP + NB - 1) // NB)]
            def phase_b(bi):
                off, nb = BLOCKS[bi]
                ns = slice(off, off + nb)
                xs = xpool.tile([128, NKC * NB], BF16, tag="xs")
                nc.sync.dma_start(
                    out=xs[:, :NKC * nb].rearrange("p (c n) -> p c n", c=NKC),
                    in_=xT[:, ns].rearrange("(c p) n -> p c n", p=128))
                pP = psB.tile([128, NB], F32, space="PSUM", tag="pP")
                for h2 in range(nb // 512):
                    hs = slice(h2 * 512, (h2 + 1) * 512)
                    for k in range(NKC):
                        nc.tensor.matmul(
                            out=pP[:, hs], lhsT=wb[:, k * HID:(k + 1) * HID],
                            rhs=xs[:, k * nb + h2 * 512:
                                   k * nb + (h2 + 1) * 512],
                            start=(k == 0), stop=False)
                    nc.tensor.matmul(
                        out=pP[:, hs], lhsT=mcS[:],
                        rhs=gT[:, off + h2 * 512:off + (h2 + 1) * 512],
                        start=False, stop=True)

                # h = relu(pP); x = min(h, 2.2).  The folded cubic coeffs are
                # large and cancelling, so the poly chunks [x, x^2, x^3] stay
                # f32 (fp32r matmuls); base + knot cubes are fp16-safe.
                xt = ew.tile([128, NB], F32R, tag="xt")
                nc.vector.tensor_scalar(out=xt[:, :nb], in0=pP[:, :nb],
                                        scalar1=0.0, scalar2=XCLAMP,
                                        op0=OP.max, op1=OP.min)
                sil = ew.tile([128, NB], FP16, tag="sil")
                nc.scalar.activation(sil[:, :nb], pP[:, :nb], AF.Silu)
                rsl = ew.tile([128, NB], FP16, tag="rsl")
                nc.vector.tensor_scalar_max(rsl[:, :nb], sil[:, :nb], 0.0)
                x2 = ew.tile([128, NB], F32R, tag="x2")
                nc.scalar.square(x2[:, :nb], xt[:, :nb])
                x3 = ew.tile([128, NB], F32R, tag="x3")
                nc.vector.scalar_tensor_tensor(
                    out=x3[:, :nb], in0=x2[:, :nb], scalar=0.0, in1=xt[:, :nb],
                    op0=OP.bypass, op1=OP.mult)
                r3s = []
                for k, tk in enumerate(KNOTS):
                    rk = ew.tile([128, NB], FP16, tag="rk")
                    nc.scalar.activation(
                        rk[:, :nb], xt[:, :nb], AF.Relu, bias=knotb[:, k:k + 1],
                        scale=scm1[:, 0:1] if k in MIRROR else 1.0)
                    r2 = ew.tile([128, NB], FP16, tag="r2")
                    nc.vector.tensor_tensor(out=r2[:, :nb], in0=rk[:, :nb],
                                            in1=rk[:, :nb], op=OP.mult)
                    r3 = ew.tile([128, NB], FP16, tag=f"r3{k}")
                    nc.vector.tensor_tensor(out=r3[:, :nb], in0=r2[:, :nb],
                                            in1=rk[:, :nb], op=OP.mult)
                    r3s.append(r3)
                pO = psO.tile([2, NB], F32, space="PSUM", tag="pO")
                # PSUM writes are capped at one bank -> half-width matmuls
                for h2 in range(nb // 512):
                    hs = slice(h2 * 512, (h2 + 1) * 512)
                    for j, ck in enumerate([xt, x2, x3]):
                        nc.tensor.matmul(out=pO[:, hs],
                                         lhsT=whR[:, 2 * j:2 * j + 2],
                                         rhs=ck[:, hs],
                                         start=(j == 0), stop=False)
                    for j, ck in enumerate([rsl] + r3s):
                        nc.tensor.matmul(out=pO[:, hs],
                                         lhsT=whB[:, 2 * j:2 * j + 2],
                                         rhs=ck[:, hs],
                                         start=False, stop=(j == 5))
                ot = ew.tile([2, NB], F32, tag="ot")
                nc.scalar.activation(ot[:, :nb], pO[:, :nb], AF.Identity,
                                     bias=q0S[:])
                nc.sync.dma_start(out=outT[:, ns], in_=ot[:, :nb])

            # ---- phase A: payload x one-hot scatter matmuls
            next_nt = [0]
            pS = [None]
            for ch in range(N_CH):
                t0 = ch * CH_T
                g0, g1 = B[t0], B[t0 + CH_T]
                cg = g1 - g0
                psb = dpool.tile([128, cg, FPAD], BF16, tag="pay")
                nc.sync.dma_start(
                    out=psb[:], in_=pay[:, g0 * FPAD:g1 * FPAD]
                    .rearrange("p (g f) -> p g f", f=FPAD))
                ohsb = dpool.tile([128, cg * 128], BF16, tag="oh")
                nc.sync.dma_start(out=ohsb[:], in_=oh[:, g0 * 128:g1 * 128])
                for t in range(t0, t0 + CH_T):
                    w = t % 4
                    if w == 0:
                        pS[0] = psA.tile([NF, 512], F32, space="PSUM",
                                         tag="pS", name="pS")
                    for gi in range(gts[t]):
                        gg = B[t] + gi - g0
                        nc.tensor.matmul(
                            out=pS[0][:, w * 128:(w + 1) * 128],
                            lhsT=psb[:, gg, 0:NF],
                            rhs=ohsb[:, gg * 128:(gg + 1) * 128],
                            start=(gi == 0), stop=(gi == gts[t] - 1))
                    if w == 3:
                        t4 = t // 4
                        nc.scalar.copy(
                            out=gT[0:NF, t4 * 512:(t4 + 1) * 512],
                            in_=pS[0][:])
                ready_cols = (ch + 1) * CH_T * 128
                while next_nt[0] < len(BLOCKS) and (
                        ch == N_CH - 1
                        or sum(BLOCKS[next_nt[0]]) <= ready_cols):
                    phase_b(next_nt[0])
                    next_nt[0] += 1

    nc.compile()
    return nc


# ----------------------------------------------------------------- entry point
def kernel(**inp):
    inp = {k: np.asarray(v) for k, v in inp.items()}
    wbigT, mcomb, head = _fold_weights(inp)
    gts, pays, ohs = _prep_edges(inp)

    if gts not in _CACHE:
        _CACHE[gts] = _build(gts)
    nc = _CACHE[gts]

    # fp16 chunks: [silu, knot cubes (mirrored for k in MIRROR)];
    # f32 chunks: [x, x^2, x^3] + q0 bias, with mirrored knots' full cubes
    # folded into the poly: w*relu(x-t)^3 = w*(x-t)^3 + w*relu(t-x)^3.
    qf = head[:, 2:10].astype(np.float64).copy()        # [128, 8] q0..q3 pairs
    for k in MIRROR:
        t = KNOTS[k]
        w = head[:, 10 + 2 * k:12 + 2 * k].astype(np.float64)
        qf[:, 0:2] += -t ** 3 * w
        qf[:, 2:4] += 3 * t ** 2 * w
        qf[:, 4:6] += -3 * t * w
        qf[:, 6:8] += w
    whb_np = np.ascontiguousarray(
        np.concatenate([head[:, 0:2], head[:, 10:20]], axis=1)
    ).astype(np.float16)
    wh3_np = np.ascontiguousarray(qf[:, 2:8]).astype(np.float32)
    q0v = qf[:, 0:2].sum(axis=0).astype(np.float32).reshape(2, 1)

    in_maps = []
    for c in range(N_CORES):
        xsh = np.zeros((KIN, NP), BF)
        xsh[:, :NSH] = inp["x_email"][c * NSH:(c + 1) * NSH].T.astype(BF)
        in_maps.append({
            "xT": xsh,
            "pay": pays[c].reshape(128, -1),
            "oh": ohs[c],
            "wbigT": wbigT.astype(BF), "mc": mcomb.astype(BF),
            "whb": whb_np, "wh3": wh3_np, "q0": q0v,
        })

    global _LAST_RESULT
    trace = os.environ.get("KERNEL_TRACE", "0") == "1"
    res = run_bass_kernel_spmd(nc, in_maps, core_ids=list(range(N_CORES)),
                               trace=trace)
    _LAST_RESULT = res
    out = np.empty((NE, 2), np.float32)
    for c in range(N_CORES):
        out[c * NSH:(c + 1) * NSH] = res.results[c]["outT"][:, :NSH].T
    return out
